# revision 1
# baseline (speedup 1.0000x reference)
"""Trainium2 Bass kernel: CodeEncoder attention pooling, histogram form.

Math per bag: out = sum_c softmax(score(idx_c))_c * table[idx_c]. Scores
depend only on the vocab id (score = W2 tanh(W1 e + b1); b2 cancels in
softmax), so with per-bag vocab counts Cnt[bag,v] (host-built from the
indices, valid codes only):

    g(v)    = exp(score_v)                      (device, score-table MLP)
    num     = Cnt @ (g*table)   [BAGS, 256]     (device, dense matmul)
    Z       = Cnt @ g           [BAGS]          (ones column of the rhs)
    out     = num / Z

This removes both data-dependent gathers (the SWDGE dma_gather and the
Q7 ap_gather dominated the old runtime). The count matrix is index prep,
computed host-side and streamed as f16.

Length-0 bags (softmax over all -1e9 -> uniform 1/64 over all 64 codes)
don't fit the weighted form: they are pooled by a small dma_gather of
their 64*32 rows + one block-diagonal mean matmul into padded output
rows; the host maps those rows back.

Sharding: data-parallel over batch, 8 batches/core on 8 cores.
Per-core pipeline: score MLP over tableT slices (PE+ACT) -> g [128,160]
via per-128-block W2 matmuls (v on psum partitions, no transposes) ->
exp -> scale rhs chunks by g (DVE) -> 160x4 matmuls accumulate
[bags<=128, 260] psum over chunks -> divide by Z column -> out.
"""

import sys

if "/opt/trn_rl_repo" not in sys.path:
    sys.path.insert(0, "/opt/trn_rl_repo")

from contextlib import ExitStack

import numpy as np

B, V, C = 64, 50, 64
NUM_CODE, D, H = 20000, 256, 128
NCORES = 8
BPC = B // NCORES          # batches per core
BAGS = BPC * V             # 400 bags per core
VP = 20480                 # padded vocab (160 chunks of 128, 40 slices of 512)
NCH = VP // 128            # 160 vocab chunks
NSL = 512                  # score-MLP slice (one f32 psum bank)
TSL = 4096                 # tableT columns per DMA
NW = D + 4                 # rhs width: 256 emb + ones col + 3 pad
MT = (128, 128, 128, 16)   # bag m-tiles
CGRP = 20                  # count-matrix chunks per DMA
ESLOT = 32                 # length-0 bag slots
ECODES = ESLOT * C         # 2048 gathered rows for the epilogue
OUTR = BAGS + ESLOT        # padded output rows

_cache = {}


def _build_program():
    import concourse.bass as bass  # noqa: F401
    import concourse.tile as tile
    from concourse import bacc, mybir

    f16 = mybir.dt.float16
    f32 = mybir.dt.float32
    f8 = mybir.dt.float8e4
    i16 = mybir.dt.int16

    nc = bacc.Bacc("TRN2", target_bir_lowering=False, debug=False,
                   num_devices=NCORES)

    table_d = nc.dram_tensor("table", [NUM_CODE, D], f16, kind="ExternalInput")
    # per-partition-contiguous: row p = [h0|h1 interleaved per TSL tile]
    tabt_d = nc.dram_tensor("tabt", [128, 2 * VP], f16, kind="ExternalInput")
    rhsc_d = nc.dram_tensor("rhsc", [128, NCH * NW], f16, kind="ExternalInput")
    cnt_d = nc.dram_tensor("cnt", [128, NCH * BAGS], f8, kind="ExternalInput")
    w1t_d = nc.dram_tensor("w1t", [D, H], f16, kind="ExternalInput")
    w2c_d = nc.dram_tensor("w2c", [H, 1], f16, kind="ExternalInput")
    b1_d = nc.dram_tensor("b1", [H, 1], f32, kind="ExternalInput")
    epool_d = nc.dram_tensor("epool", [128, 16 * 32], f16, kind="ExternalInput")
    egidx_d = nc.dram_tensor("egidx", [128, ECODES // 16], i16,
                             kind="ExternalInput")
    out_d = nc.dram_tensor("out", [OUTR, D], f32, kind="ExternalOutput")

    with tile.TileContext(nc) as tc, ExitStack() as ctx:
        const = ctx.enter_context(tc.tile_pool(name="const", bufs=1))
        tabp = ctx.enter_context(tc.tile_pool(name="tabp", bufs=3))
        hp = ctx.enter_context(tc.tile_pool(name="hp", bufs=3))
        cp = ctx.enter_context(tc.tile_pool(name="cp", bufs=3))
        tgp = ctx.enter_context(tc.tile_pool(name="tgp", bufs=8))
        outp = ctx.enter_context(tc.tile_pool(name="outp", bufs=2))
        php = ctx.enter_context(tc.tile_pool(name="ph", bufs=2, space="PSUM"))
        gp = ctx.enter_context(tc.tile_pool(name="gp", bufs=1, space="PSUM"))
        mp = ctx.enter_context(tc.tile_pool(name="mp", bufs=1, space="PSUM"))
        epp = ctx.enter_context(tc.tile_pool(name="epp", bufs=1, space="PSUM"))

        # --- constants ---
        w1t_sb = const.tile([128, 2, H], f16)
        nc.sync.dma_start(w1t_sb[:, 0, :], w1t_d.ap()[0:128, :])
        nc.sync.dma_start(w1t_sb[:, 1, :], w1t_d.ap()[128:256, :])
        w2c_sb = const.tile([H, 1], f16)
        nc.sync.dma_start(w2c_sb[:], w2c_d.ap())
        b1_sb = const.tile([H, 1], f32)
        nc.sync.dma_start(b1_sb[:], b1_d.ap())
        epool_sb = const.tile([128, 16, 32], f16)
        nc.sync.dma_start(epool_sb[:].rearrange("p a b -> p (a b)"),
                          epool_d.ap())
        egidx_sb = const.tile([128, ECODES // 16], i16)
        nc.sync.dma_start(egidx_sb[:], egidx_d.ap())

        # --- score table: g = exp(W2 tanh(W1 tabT + b1)), v on partitions ---
        rhsc_sb = const.tile([128, NCH, NW], f16)

        # --- fused score + main loop: one continuous PE stream.
        # Per tableT tile: MLP -> g for its 32 chunks, then immediately the
        # 32 main-matmul chunks; the next tile's DMA overlaps.
        mtiles = []
        o = 0
        for m in MT:
            mtiles.append((o, m))
            o += m
        mps = [mp.tile([128, NW], f32, name=f"mps{t}", tag=f"mps{t}")
               for t in range(len(MT))]
        g_ps = gp.tile([128, NCH], f32)
        g_sb = const.tile([128, NCH], f32)
        TCH = TSL // 128  # chunks per tableT tile
        RP = 16           # rhs chunks per SWDGE piece (2 pieces per tile)
        ct = None
        egat = const.tile([128, ECODES // 128, D], f16)
        for ti in range(VP // TSL):
            # all bulk uploads ride the SWDGE ring, hand-interleaved so the
            # tableT tile the PE needs next is always at the ring head
            tab_t = tabp.tile([128, 2, TSL], f16)
            nc.gpsimd.dma_start(
                tab_t[:].rearrange("p a b -> p (a b)"),
                tabt_d.ap()[:, ti * 2 * TSL:(ti + 1) * 2 * TSL])
            for h in range(2):
                a = (2 * ti + h) * RP
                nc.gpsimd.dma_start(
                    rhsc_sb[:, a:a + RP, :].rearrange("p a b -> p (a b)"),
                    rhsc_d.ap()[:, a * NW:(a + RP) * NW])
            if ti == 3:
                # length-0 epilogue gather: mid-loop so its packets drain
                # during the main stream, not after it
                for k in range(ECODES // 1024):
                    nc.gpsimd.dma_gather(
                        egat[:, k * 8:(k + 1) * 8, :], table_d.ap(),
                        egidx_sb[:, k * 64:(k + 1) * 64], 1024, 1024, D)
            for sub in range(TSL // NSL):
                ssl = slice(sub * NSL, (sub + 1) * NSL)
                ph = php.tile([128, NSL], f32)
                nc.tensor.matmul(ph[:], w1t_sb[:, 0, :], tab_t[:, 0, ssl],
                                 start=True, stop=False)
                nc.tensor.matmul(ph[:], w1t_sb[:, 1, :], tab_t[:, 1, ssl],
                                 start=False, stop=True)
                h1 = hp.tile([128, NSL], f16)
                nc.scalar.activation(h1[:], ph[:],
                                     mybir.ActivationFunctionType.Tanh,
                                     bias=b1_sb[:], scale=1.0)
                for k in range(NSL // 128):
                    j = (ti * (TSL // NSL) + sub) * (NSL // 128) + k
                    nc.tensor.matmul(g_ps[:, j:j + 1],
                                     h1[:, k * 128:(k + 1) * 128], w2c_sb[:],
                                     start=True, stop=True)
            gsl = slice(ti * TCH, (ti + 1) * TCH)
            nc.scalar.activation(g_sb[:, gsl], g_ps[:, gsl],
                                 mybir.ActivationFunctionType.Exp)
            for jj in range(TCH):
                j = ti * TCH + jj
                if j % CGRP == 0:
                    ct = cp.tile([128, CGRP, BAGS], f8)
                    nc.sync.dma_start(
                        ct[:].rearrange("p a b -> p (a b)"),
                        cnt_d.ap()[:, j * BAGS:(j + CGRP) * BAGS])
                tg = tgp.tile([128, NW], f16)
                nc.vector.tensor_scalar(tg[:], rhsc_sb[:, j, :],
                                        g_sb[:, j:j + 1], None,
                                        mybir.AluOpType.mult)
                for t, (o, m) in enumerate(mtiles):
                    nc.tensor.matmul(mps[t][0:m, :], ct[:, j % CGRP, o:o + m],
                                     tg[:], start=(j == 0), stop=(j == NCH - 1))

        # --- length-0 epilogue: mean over 64 codes per slot ---
        eps = epp.tile([32, D], f32)
        for gb in range(16):
            nc.tensor.matmul(eps[:], epool_sb[:, gb, :], egat[:, gb, :],
                             start=(gb == 0), stop=(gb == 15))
        eout = outp.tile([32, D], f32)
        nc.vector.tensor_copy(eout[:], eps[:])
        nc.sync.dma_start(out_d.ap()[BAGS:OUTR, :], eout[:])

        # --- normalize and store ---
        for t, (o, m) in enumerate(mtiles):
            rz = hp.tile([128, 1], f32, tag=f"rz{t}")
            nc.vector.reciprocal(rz[0:m], mps[t][0:m, D:D + 1])
            osb = outp.tile([128, D], f32)
            nc.vector.tensor_scalar(osb[0:m, :], mps[t][0:m, 0:D], rz[0:m],
                                    None, mybir.AluOpType.mult)
            nc.sync.dma_start(out_d.ap()[o:o + m, :], osb[0:m, :])

    nc.compile()
    return nc


def _wrap16(idx_flat):
    n = idx_flat.shape[0]
    return idx_flat.reshape(n // 16, 16).T.copy()


def _prep_shared(embed_table, W1, b1, W2):
    tab16 = embed_table.astype(np.float16)                    # [20000, 256]
    tabt = np.zeros((D, VP), np.float16)
    tabt[:, :NUM_CODE] = tab16.T
    # per-partition-contiguous interleave: [128, tiles, half, TSL]
    tabt = np.ascontiguousarray(
        tabt.reshape(2, 128, VP // TSL, TSL).transpose(1, 2, 0, 3)
    ).reshape(128, 2 * VP)
    rhsc = np.zeros((VP, NW), np.float16)
    rhsc[:NUM_CODE, :D] = tab16
    rhsc[:, D] = 1.0
    rhsc = np.ascontiguousarray(
        rhsc.reshape(NCH, 128, NW).transpose(1, 0, 2)).reshape(128, NCH * NW)
    w1t = np.ascontiguousarray(W1.astype(np.float16).T)       # [256, 128]
    w2c = np.ascontiguousarray(W2.astype(np.float16).reshape(H, 1))
    b1c = np.ascontiguousarray(b1.astype(np.float32).reshape(H, 1))
    epool = np.zeros((128, 16, 32), np.float16)
    for g in range(16):
        epool[0:64, g, 2 * g] = 1.0 / C
        epool[64:128, g, 2 * g + 1] = 1.0 / C
    epool = epool.reshape(128, 16 * 32)
    return dict(table=tab16, tabt=tabt, rhsc=rhsc, w1t=w1t, w2c=w2c, b1=b1c,
                epool=epool)


def build_in_maps(input_code, length_code, shared):
    in_maps = []
    len0_lists = []
    for core in range(NCORES):
        bs = slice(core * BPC, (core + 1) * BPC)
        codes = input_code[bs].reshape(BAGS, C).astype(np.int64)
        lens = length_code[bs].reshape(BAGS).astype(np.int64)
        valid = np.arange(C)[None, :] < lens[:, None]          # [400, 64]
        bb, cc = np.nonzero(valid)
        cnt = np.zeros((VP, BAGS), np.float32)
        np.add.at(cnt, (codes[bb, cc], bb), 1.0)
        import ml_dtypes
        cnt = np.ascontiguousarray(
            cnt.reshape(NCH, 128, BAGS).transpose(1, 0, 2)
        ).astype(ml_dtypes.float8_e4m3).reshape(128, NCH * BAGS)
        len0 = np.nonzero(lens == 0)[0][:ESLOT]
        ecodes = np.zeros(ECODES, np.int16)
        for s, b in enumerate(len0):
            ecodes[s * C:(s + 1) * C] = codes[b]
        egidx = np.tile(_wrap16(ecodes), (8, 1))               # [128, 128]
        len0_lists.append(len0)
        in_maps.append(dict(shared, cnt=cnt, egidx=egidx))
    return in_maps, len0_lists


def kernel(input_code, length_code, embed_table, W1, b1, W2, b2):
    from concourse.bass_utils import run_bass_kernel_spmd

    if "nc" not in _cache:
        _cache["nc"] = _build_program()
    nc = _cache["nc"]

    shared = _prep_shared(np.asarray(embed_table), np.asarray(W1),
                          np.asarray(b1), np.asarray(W2))
    input_code = np.asarray(input_code)
    length_code = np.asarray(length_code)

    in_maps, len0_lists = build_in_maps(input_code, length_code, shared)
    res = run_bass_kernel_spmd(nc, in_maps, core_ids=list(range(NCORES)))
    outs = []
    for c in range(NCORES):
        full = res.results[c]["out"]
        o = full[:BAGS].copy()
        for s, b in enumerate(len0_lists[c]):
            o[b] = full[BAGS + s]
        outs.append(o.reshape(BPC, V, D))
    return np.concatenate(outs, axis=0)



# revision 2
# speedup vs baseline: 1.0221x; 1.0221x over previous
"""Trainium2 Bass kernel: CodeEncoder attention pooling, vocab-sharded
histogram form with cross-core ReduceScatter.

Math per bag: out = sum_c softmax(score(idx_c))_c * table[idx_c]. Scores
depend only on the vocab id (score = W2 tanh(W1 e + b1); b2 cancels in
softmax), so with per-bag vocab counts Cnt[v, bag] (host-built):

    g(v) = exp(score_v)                    (device, score-table MLP)
    num  = (g*table)^T @ Cnt  [bags, 257]  (dense matmul, ones col -> Z)
    out  = num / Z

Sharding: VOCAB-sharded. Core k owns vocab slice [2560k, 2560k+2560):
it runs the score MLP on its slice only (1/8 the table traffic and
MLP flops of the batch-parallel form) and accumulates partial num/Z
for ALL 3200 bags over its slice. Partials are bf16-rounded and
ReduceScattered (5 groups, overlapped with compute) so core k ends up
with its own 400 bags, which it normalizes and stores.

Length-0 bags (softmax over all-masked -> uniform mean of all 64
codes) ride the same matmuls as 32 extra "slot" columns per core whose
counts are the full-64-code histogram and whose rhs is the RAW
(unscaled) table; a final small ReduceScatter returns them. The host
maps slots back onto their bags.

Main matmul path selectable: f16 rhs (1 cyc/col) or fp8e4m3 hi+lo
DoubleRow pairs (2 chunks + both hi/lo terms in 2x129 cyc -> ~2x fewer
PE cycles; hi+lo recovers ~f16 accuracy).
"""

import sys

if "/opt/trn_rl_repo" not in sys.path:
    sys.path.insert(0, "/opt/trn_rl_repo")

from contextlib import ExitStack

import numpy as np

B, V, C = 64, 50, 64
NUM_CODE, D, H = 20000, 256, 128
NCORES = 8
BPC = B // NCORES          # batches per core
BAGS = BPC * V             # 400 bags owned per core
GBAGS = B * V              # 3200 global bags
VP = 20480                 # padded vocab
VSL = VP // NCORES         # 2560 vocab per core
NCH = VSL // 128           # 20 vocab chunks per core
NPAIR = NCH // 2           # 10 DoubleRow chunk pairs
NSL = 512                  # score-MLP slice (one f32 psum bank)
NW = D + 2                 # rhs width: 256 emb + ones col + pad
SLOTS = 32                 # len-0 slots per core
NSLOT = SLOTS * NCORES     # 256 slot columns (2 tiles)
NBT = GBAGS // 128         # 25 bag tiles
NT = NBT + NSLOT // 128    # 27 matmul tiles
RSG = 5                    # ReduceScatter groups
TPG = NBT // RSG           # 5 bag tiles per group
RROWS = GBAGS // RSG       # 640 partial rows per group
OROWS = RROWS // NCORES    # 80 owned rows per group
ORT = BAGS + SLOTS         # 432 output rows

USE_DR = False             # fp8 hi/lo DoubleRow main matmul

_cache = {}


def _build_program():
    import concourse.bass as bass  # noqa: F401
    import concourse.tile as tile
    from concourse import bacc, mybir

    f16 = mybir.dt.float16
    f32 = mybir.dt.float32
    bf16 = mybir.dt.bfloat16
    f8 = mybir.dt.float8e4

    nc = bacc.Bacc("TRN2", target_bir_lowering=False, debug=False,
                   num_devices=NCORES)

    tabt_d = nc.dram_tensor("tabt", [128, 2 * VSL], f16, kind="ExternalInput")
    rhsc_d = nc.dram_tensor("rhsc", [128, NCH * NW], f16, kind="ExternalInput")
    cnt_d = nc.dram_tensor("cnt", [128, NT * NCH * 128], f8,
                           kind="ExternalInput")
    w1t_d = nc.dram_tensor("w1t", [D, H], f16, kind="ExternalInput")
    w2c_d = nc.dram_tensor("w2c", [H, 1], f16, kind="ExternalInput")
    b1_d = nc.dram_tensor("b1", [H, 1], f32, kind="ExternalInput")
    out_d = nc.dram_tensor("out", [ORT, D], f32, kind="ExternalOutput")

    groups = [list(range(NCORES))]

    with tile.TileContext(nc) as tc, ExitStack() as ctx:
        const = ctx.enter_context(tc.tile_pool(name="const", bufs=1))
        cntp = ctx.enter_context(tc.tile_pool(name="cntp", bufs=6))
        hp = ctx.enter_context(tc.tile_pool(name="hp", bufs=2))
        obp = ctx.enter_context(tc.tile_pool(name="obp", bufs=3))
        normp = ctx.enter_context(tc.tile_pool(name="normp", bufs=2))
        php = ctx.enter_context(tc.tile_pool(name="ph", bufs=2, space="PSUM"))
        gpp = ctx.enter_context(tc.tile_pool(name="gp", bufs=1, space="PSUM"))
        psp = ctx.enter_context(tc.tile_pool(name="ps", bufs=2, space="PSUM"))
        dram = ctx.enter_context(tc.tile_pool(name="dram", bufs=1,
                                              space="DRAM"))

        part_t = dram.tile([GBAGS, NW], bf16)
        spart_t = dram.tile([NSLOT, NW], bf16)
        rsout_t = dram.tile([BAGS, NW], bf16)
        rsslot_t = dram.tile([SLOTS, NW], bf16)

        # --- constants ---
        w1t_sb = const.tile([128, 2, H], f16)
        nc.gpsimd.dma_start(w1t_sb[:, 0, :], w1t_d.ap()[0:128, :])
        nc.gpsimd.dma_start(w1t_sb[:, 1, :], w1t_d.ap()[128:256, :])
        w2c_sb = const.tile([H, 1], f16)
        nc.gpsimd.dma_start(w2c_sb[:], w2c_d.ap())
        b1_sb = const.tile([H, 1], f32)
        nc.gpsimd.dma_start(b1_sb[:], b1_d.ap())
        tabt_sb = const.tile([128, 2, VSL], f16)
        nc.gpsimd.dma_start(tabt_sb[:].rearrange("p a b -> p (a b)"),
                            tabt_d.ap())
        rhsc_sb = const.tile([128, NCH, NW], f16)
        nc.gpsimd.dma_start(rhsc_sb[:].rearrange("p a b -> p (a b)"),
                            rhsc_d.ap())

        g_sb = const.tile([128, NCH], f32)
        tg_sb = const.tile([128, NCH, NW], f16)
        g_ps = gpp.tile([128, NCH], f32)
        if USE_DR:
            hi_sb = const.tile([128, NPAIR, 2, NW], f8, tag="hi")
            lo_sb = const.tile([128, NPAIR, 2, NW], f8, tag="lo")
            rhi_sb = const.tile([128, NPAIR, 2, NW], f8, tag="rhi")
            rlo_sb = const.tile([128, NPAIR, 2, NW], f8, tag="rlo")

        # --- score MLP over the vocab slice, then per-chunk rhs scaling ---
        for s in range(VSL // NSL):
            ssl = slice(s * NSL, (s + 1) * NSL)
            ph = php.tile([128, NSL], f32)
            nc.tensor.matmul(ph[:], w1t_sb[:, 0, :], tabt_sb[:, 0, ssl],
                             start=True, stop=False)
            nc.tensor.matmul(ph[:], w1t_sb[:, 1, :], tabt_sb[:, 1, ssl],
                             start=False, stop=True)
            h1 = hp.tile([128, NSL], f16)
            nc.scalar.activation(h1[:], ph[:],
                                 mybir.ActivationFunctionType.Tanh,
                                 bias=b1_sb[:], scale=1.0)
            for k in range(NSL // 128):
                j = s * (NSL // 128) + k
                nc.tensor.matmul(g_ps[:, j:j + 1],
                                 h1[:, k * 128:(k + 1) * 128], w2c_sb[:],
                                 start=True, stop=True)
            jsl = slice(s * (NSL // 128), (s + 1) * (NSL // 128))
            nc.scalar.activation(g_sb[:, jsl], g_ps[:, jsl],
                                 mybir.ActivationFunctionType.Exp)
            for k in range(NSL // 128):
                j = s * (NSL // 128) + k
                nc.vector.tensor_scalar(tg_sb[:, j, :], rhsc_sb[:, j, :],
                                        g_sb[:, j:j + 1], None,
                                        mybir.AluOpType.mult)
                if USE_DR:
                    q, i = divmod(j, 2)
                    nc.vector.tensor_copy(hi_sb[:, q, i, :], tg_sb[:, j, :])
                    nc.vector.tensor_tensor(lo_sb[:, q, i, :], tg_sb[:, j, :],
                                            hi_sb[:, q, i, :],
                                            mybir.AluOpType.subtract)

        if USE_DR:
            for j in range(NCH):
                q, i = divmod(j, 2)
                nc.vector.tensor_copy(rhi_sb[:, q, i, :], rhsc_sb[:, j, :])
                nc.vector.tensor_tensor(rlo_sb[:, q, i, :], rhsc_sb[:, j, :],
                                        rhi_sb[:, q, i, :],
                                        mybir.AluOpType.subtract)

        # --- main loop: per bag tile, contract all 20 chunks, K-contiguous ---
        for t in range(NT):
            ct = cntp.tile([128, NCH, 128], f8)
            nc.sync.dma_start(ct[:].rearrange("p a b -> p (a b)"),
                              cnt_d.ap()[:, t * NCH * 128:(t + 1) * NCH * 128])
            ps = psp.tile([128, NW], f32)
            raw = t >= NBT
            if USE_DR:
                hsrc = rhi_sb if raw else hi_sb
                lsrc = rlo_sb if raw else lo_sb
                for q in range(NPAIR):
                    nc.tensor.matmul(ps[:], ct[:, 2 * q:2 * q + 2, :],
                                     hsrc[:, q, :, :], start=(q == 0),
                                     stop=False,
                                     perf_mode=mybir.MatmulPerfMode.DoubleRow)
                    nc.tensor.matmul(ps[:], ct[:, 2 * q:2 * q + 2, :],
                                     lsrc[:, q, :, :], start=False,
                                     stop=(q == NPAIR - 1),
                                     perf_mode=mybir.MatmulPerfMode.DoubleRow)
            else:
                src = rhsc_sb if raw else tg_sb
                for j in range(NCH):
                    nc.tensor.matmul(ps[:], ct[:, j, :], src[:, j, :],
                                     start=(j == 0), stop=(j == NCH - 1))
            ob = obp.tile([128, NW], bf16)
            nc.vector.tensor_copy(ob[:], ps[:])
            if raw:
                r0 = (t - NBT) * 128
                nc.gpsimd.dma_start(spart_t[r0:r0 + 128, :], ob[:])
            else:
                nc.gpsimd.dma_start(part_t[t * 128:(t + 1) * 128, :], ob[:])
                if t % TPG == TPG - 1:
                    gi = t // TPG
                    nc.gpsimd.collective_compute(
                        "ReduceScatter", mybir.AluOpType.add, groups,
                        ins=[part_t[gi * RROWS:(gi + 1) * RROWS, :]],
                        outs=[rsout_t[gi * OROWS:(gi + 1) * OROWS, :]])
        nc.gpsimd.collective_compute(
            "ReduceScatter", mybir.AluOpType.add, groups,
            ins=[spart_t[:, :]], outs=[rsslot_t[:, :]])

        # --- readback, normalize, store ---
        parts = [(rsout_t, 0, 128, 0), (rsout_t, 128, 128, 128),
                 (rsout_t, 256, 128, 256), (rsout_t, 384, 16, 384),
                 (rsslot_t, 0, SLOTS, BAGS)]
        for srcT, off, m, oo in parts:
            it = normp.tile([128, NW], bf16, tag="it")
            nc.sync.dma_start(it[0:m, :], srcT[off:off + m, :])
            rz = normp.tile([128, 1], f32, tag="rz")
            nc.vector.reciprocal(rz[0:m], it[0:m, D:D + 1])
            osb = normp.tile([128, D], f32, tag="osb")
            nc.vector.tensor_scalar(osb[0:m, :], it[0:m, 0:D], rz[0:m],
                                    None, mybir.AluOpType.mult)
            nc.sync.dma_start(out_d.ap()[oo:oo + m, :], osb[0:m, :])

    nc.compile()
    return nc


def _prep_shared(embed_table, W1, b1, W2):
    """Per-core-sliceable views of the table + tiny MLP weights."""
    t16 = embed_table.astype(np.float16)                      # [20000, 256]
    tabt = np.zeros((D, VP), np.float16)
    tabt[:, :NUM_CODE] = t16.T
    rhsc = np.zeros((VP, NW), np.float16)
    rhsc[:NUM_CODE, :D] = t16
    rhsc[:NUM_CODE, D] = 1.0
    w1t = np.ascontiguousarray(W1.astype(np.float16).T)       # [256, 128]
    w2c = np.ascontiguousarray(W2.astype(np.float16).reshape(H, 1))
    b1c = np.ascontiguousarray(b1.astype(np.float32).reshape(H, 1))
    return dict(tabt=tabt, rhsc=rhsc, w1t=w1t, w2c=w2c, b1=b1c)


def build_in_maps(input_code, length_code, shared):
    import ml_dtypes

    codes = input_code.reshape(GBAGS, C).astype(np.int64)
    lens = length_code.reshape(GBAGS).astype(np.int64)

    # global column order: 5 RS groups x (8 cores x 80 bags), then slots
    gb = np.arange(GBAGS)
    k = gb // BAGS
    i = gb % BAGS
    col_of_gb = (i // OROWS) * RROWS + k * OROWS + (i % OROWS)

    NCOL = GBAGS + NSLOT
    cnt = np.zeros((VP, NCOL), np.float32)
    valid = np.arange(C)[None, :] < lens[:, None]
    bb, cc = np.nonzero(valid)
    np.add.at(cnt, (codes[bb, cc], col_of_gb[bb]), 1.0)

    len0_lists = []
    for core in range(NCORES):
        len0 = np.nonzero(lens[core * BAGS:(core + 1) * BAGS] == 0)[0][:SLOTS]
        len0_lists.append(len0)
        for s, b in enumerate(len0):
            np.add.at(cnt, (codes[core * BAGS + b],
                            GBAGS + core * SLOTS + s), 1.0)

    in_maps = []
    for core in range(NCORES):
        vs = slice(core * VSL, (core + 1) * VSL)
        cslice = cnt[vs]                                      # [2560, 3456]
        cntl = np.ascontiguousarray(
            cslice.reshape(NCH, 128, NT, 128).transpose(1, 2, 0, 3)
        ).astype(ml_dtypes.float8_e4m3).reshape(128, NT * NCH * 128)
        tabtc = np.ascontiguousarray(
            shared["tabt"][:, vs].reshape(2, 128, VSL).transpose(1, 0, 2)
        ).reshape(128, 2 * VSL)
        rhscc = np.ascontiguousarray(
            shared["rhsc"][vs].reshape(NCH, 128, NW).transpose(1, 0, 2)
        ).reshape(128, NCH * NW)
        in_maps.append(dict(tabt=tabtc, rhsc=rhscc, cnt=cntl,
                            w1t=shared["w1t"], w2c=shared["w2c"],
                            b1=shared["b1"]))
    return in_maps, len0_lists


def kernel(input_code, length_code, embed_table, W1, b1, W2, b2):
    from concourse.bass_utils import run_bass_kernel_spmd

    if "nc" not in _cache:
        _cache["nc"] = _build_program()
    nc = _cache["nc"]

    shared = _prep_shared(np.asarray(embed_table), np.asarray(W1),
                          np.asarray(b1), np.asarray(W2))
    in_maps, len0_lists = build_in_maps(np.asarray(input_code),
                                        np.asarray(length_code), shared)
    res = run_bass_kernel_spmd(nc, in_maps, core_ids=list(range(NCORES)))
    outs = []
    for c in range(NCORES):
        full = res.results[c]["out"]
        o = full[:BAGS].copy()
        for s, b in enumerate(len0_lists[c]):
            o[b] = full[BAGS + s]
        outs.append(o.reshape(BPC, V, D))
    return np.concatenate(outs, axis=0)


# revision 7
# speedup vs baseline: 1.3013x; 1.2732x over previous
"""Trainium2 Bass kernel: CodeEncoder attention pooling, vocab-sharded
histogram form with cross-core ReduceScatter.

Math per bag: out = sum_c softmax(score(idx_c))_c * table[idx_c]. Scores
depend only on the vocab id (score = W2 tanh(W1 e + b1); b2 cancels in
softmax), so with per-bag vocab counts Cnt[v, bag] (host-built):

    g(v) = exp(score_v)                    (device, score-table MLP)
    num  = (g*table)^T @ Cnt  [bags, 257]  (dense matmul, ones col -> Z)
    out  = num / Z

Sharding: VOCAB-sharded. Core k owns vocab slice [2560k, 2560k+2560):
it runs the score MLP on its slice only (1/8 the table traffic and
MLP flops of the batch-parallel form) and accumulates partial num/Z
for ALL 3200 bags over its slice. Partials are bf16-rounded and
ReduceScattered (5 groups, overlapped with compute) so core k ends up
with its own 400 bags, which it normalizes and stores.

Length-0 bags (softmax over all-masked -> uniform mean of all 64
codes) ride the same matmuls as 32 extra "slot" columns per core whose
counts are the full-64-code histogram and whose rhs is the RAW
(unscaled) table; a final small ReduceScatter returns them. The host
maps slots back onto their bags.

Main matmul path selectable: f16 rhs (1 cyc/col) or fp8e4m3 hi+lo
DoubleRow pairs (2 chunks + both hi/lo terms in 2x129 cyc -> ~2x fewer
PE cycles; hi+lo recovers ~f16 accuracy).
"""

import sys

if "/opt/trn_rl_repo" not in sys.path:
    sys.path.insert(0, "/opt/trn_rl_repo")

from contextlib import ExitStack

import numpy as np

B, V, C = 64, 50, 64
NUM_CODE, D, H = 20000, 256, 128
NCORES = 8
BPC = B // NCORES          # batches per core
BAGS = BPC * V             # 400 bags owned per core
GBAGS = B * V              # 3200 global bags
VP = 20480                 # padded vocab
VSL = VP // NCORES         # 2560 vocab per core
NCH = VSL // 128           # 20 vocab chunks per core
NPAIR = NCH // 2           # 10 DoubleRow chunk pairs
NSL = 512                  # score-MLP slice (one f32 psum bank)
NW = D + 2                 # rhs width: 256 emb + ones col + pad
SLOTS = 32                 # len-0 slots per core
NSLOT = SLOTS * NCORES     # 256 slot columns (2 tiles)
NST = NSLOT // 128         # 2 slot tiles (computed first)
NBT = GBAGS // 128         # 25 bag tiles
NT = NBT + NST             # 27 matmul tiles
ORT = BAGS + SLOTS         # 432 output rows

USE_DR = False             # fp8 hi/lo DoubleRow main matmul

_cache = {}


def _build_program():
    import concourse.bass as bass  # noqa: F401
    import concourse.tile as tile
    from concourse import bacc, mybir

    f16 = mybir.dt.float16
    f32 = mybir.dt.float32
    bf16 = mybir.dt.bfloat16
    f8 = mybir.dt.float8e4

    nc = bacc.Bacc("TRN2", target_bir_lowering=False, debug=False,
                   num_devices=NCORES)

    tabt_d = nc.dram_tensor("tabt", [128, 2 * VSL], f16, kind="ExternalInput")
    rhsc_d = nc.dram_tensor("rhsc", [128, NCH * NW], f16, kind="ExternalInput")
    cnt_d = nc.dram_tensor("cnt", [128, NT * NCH * 128], f8,
                           kind="ExternalInput")
    w1t_d = nc.dram_tensor("w1t", [D, H], f16, kind="ExternalInput")
    w2c_d = nc.dram_tensor("w2c", [H, 1], f16, kind="ExternalInput")
    b1_d = nc.dram_tensor("b1", [H, 1], f32, kind="ExternalInput")
    out_d = nc.dram_tensor("out", [ORT, D], f32, kind="ExternalOutput")

    groups = [list(range(NCORES))]

    with tile.TileContext(nc) as tc, ExitStack() as ctx:
        const = ctx.enter_context(tc.tile_pool(name="const", bufs=1))
        cntp = ctx.enter_context(tc.tile_pool(name="cntp", bufs=6))
        hp = ctx.enter_context(tc.tile_pool(name="hp", bufs=2))
        obp = ctx.enter_context(tc.tile_pool(name="obp", bufs=3))
        normp = ctx.enter_context(tc.tile_pool(name="normp", bufs=2))
        php = ctx.enter_context(tc.tile_pool(name="ph", bufs=2, space="PSUM"))
        gpp = ctx.enter_context(tc.tile_pool(name="gp", bufs=1, space="PSUM"))
        psp = ctx.enter_context(tc.tile_pool(name="ps", bufs=2, space="PSUM"))
        dram = ctx.enter_context(tc.tile_pool(name="dram", bufs=1,
                                              space="DRAM"))

        part_t = dram.tile([GBAGS, NW], bf16)
        spart_t = dram.tile([NSLOT, NW], bf16)
        rsout_t = dram.tile([BAGS, NW], bf16)
        rsslot_t = dram.tile([SLOTS, NW], bf16)

        # --- constants; tabt/rhsc interleaved in MLP-slice-sized pieces so
        # the PE can start ~2us in instead of waiting for the full upload ---
        w1t_sb = const.tile([128, 2, H], f16)
        nc.gpsimd.dma_start(w1t_sb[:, 0, :], w1t_d.ap()[0:128, :])
        nc.gpsimd.dma_start(w1t_sb[:, 1, :], w1t_d.ap()[128:256, :])
        w2c_sb = const.tile([H, 1], f16)
        nc.gpsimd.dma_start(w2c_sb[:], w2c_d.ap())
        b1_sb = const.tile([H, 1], f32)
        nc.gpsimd.dma_start(b1_sb[:], b1_d.ap())
        tabt_sb = const.tile([128, 2, VSL], f16)
        rhsc_sb = const.tile([128, NCH, NW], f16)
        CPS = NCH // (VSL // NSL)  # rhs chunks per MLP slice
        for s in range(VSL // NSL):
            ssl = slice(s * NSL, (s + 1) * NSL)
            nc.gpsimd.dma_start(
                tabt_sb[:, :, ssl],
                tabt_d.ap()[:, :].rearrange("p (a b) -> p a b", a=2)[:, :, ssl])
            nc.gpsimd.dma_start(
                rhsc_sb[:, s * CPS:(s + 1) * CPS, :].rearrange(
                    "p a b -> p (a b)"),
                rhsc_d.ap()[:, s * CPS * NW:(s + 1) * CPS * NW])

        g_sb = const.tile([128, NCH], f32)
        tg_sb = const.tile([128, NCH, NW], f16)
        g_ps = gpp.tile([128, NCH], f32)
        if USE_DR:
            hi_sb = const.tile([128, NPAIR, 2, NW], f8, tag="hi")
            lo_sb = const.tile([128, NPAIR, 2, NW], f8, tag="lo")
            rhi_sb = const.tile([128, NPAIR, 2, NW], f8, tag="rhi")
            rlo_sb = const.tile([128, NPAIR, 2, NW], f8, tag="rlo")

        # --- score MLP over the vocab slice, then per-chunk rhs scaling ---
        for s in range(VSL // NSL):
            ssl = slice(s * NSL, (s + 1) * NSL)
            ph = php.tile([128, NSL], f32)
            nc.tensor.matmul(ph[:], w1t_sb[:, 0, :], tabt_sb[:, 0, ssl],
                             start=True, stop=False)
            nc.tensor.matmul(ph[:], w1t_sb[:, 1, :], tabt_sb[:, 1, ssl],
                             start=False, stop=True)
            h1 = hp.tile([128, NSL], f16)
            nc.scalar.activation(h1[:], ph[:],
                                 mybir.ActivationFunctionType.Tanh,
                                 bias=b1_sb[:], scale=1.0)
            for k in range(NSL // 128):
                j = s * (NSL // 128) + k
                nc.tensor.matmul(g_ps[:, j:j + 1],
                                 h1[:, k * 128:(k + 1) * 128], w2c_sb[:],
                                 start=True, stop=True)
            jsl = slice(s * (NSL // 128), (s + 1) * (NSL // 128))
            nc.scalar.activation(g_sb[:, jsl], g_ps[:, jsl],
                                 mybir.ActivationFunctionType.Exp)
            for k in range(NSL // 128):
                j = s * (NSL // 128) + k
                nc.vector.tensor_scalar(tg_sb[:, j, :], rhsc_sb[:, j, :],
                                        g_sb[:, j:j + 1], None,
                                        mybir.AluOpType.mult)
                if USE_DR:
                    q, i = divmod(j, 2)
                    nc.vector.tensor_copy(hi_sb[:, q, i, :], tg_sb[:, j, :])
                    nc.vector.tensor_tensor(lo_sb[:, q, i, :], tg_sb[:, j, :],
                                            hi_sb[:, q, i, :],
                                            mybir.AluOpType.subtract)

        if USE_DR:
            for j in range(NCH):
                q, i = divmod(j, 2)
                nc.vector.tensor_copy(rhi_sb[:, q, i, :], rhsc_sb[:, j, :])
                nc.vector.tensor_tensor(rlo_sb[:, q, i, :], rhsc_sb[:, j, :],
                                        rhi_sb[:, q, i, :],
                                        mybir.AluOpType.subtract)

        # --- main loop: slot tiles first (raw rhs; their RS overlaps the
        # whole bag stream), then 25 bag tiles; K-contiguous per tile ---
        for t in range(NT):
            ct = cntp.tile([128, NCH, 128], f8)
            nc.sync.dma_start(ct[:].rearrange("p a b -> p (a b)"),
                              cnt_d.ap()[:, t * NCH * 128:(t + 1) * NCH * 128])
            ps = psp.tile([128, NW], f32)
            raw = t < NST
            if USE_DR:
                hsrc = rhi_sb if raw else hi_sb
                lsrc = rlo_sb if raw else lo_sb
                for q in range(NPAIR):
                    nc.tensor.matmul(ps[:], ct[:, 2 * q:2 * q + 2, :],
                                     hsrc[:, q, :, :], start=(q == 0),
                                     stop=False,
                                     perf_mode=mybir.MatmulPerfMode.DoubleRow)
                    nc.tensor.matmul(ps[:], ct[:, 2 * q:2 * q + 2, :],
                                     lsrc[:, q, :, :], start=False,
                                     stop=(q == NPAIR - 1),
                                     perf_mode=mybir.MatmulPerfMode.DoubleRow)
            else:
                src = rhsc_sb if raw else tg_sb
                for j in range(NCH):
                    nc.tensor.matmul(ps[:], ct[:, j, :], src[:, j, :],
                                     start=(j == 0), stop=(j == NCH - 1))
            ob = obp.tile([128, NW], bf16)
            nc.vector.tensor_copy(ob[:], ps[:])
            if raw:
                nc.gpsimd.dma_start(spart_t[t * 128:(t + 1) * 128, :], ob[:])
            else:
                r0 = (t - NST) * 128
                nc.gpsimd.dma_start(part_t[r0:r0 + 128, :], ob[:])
        nc.gpsimd.collective_compute(
            "ReduceScatter", mybir.AluOpType.add, groups,
            ins=[spart_t[:, :]], outs=[rsslot_t[:, :]])
        nc.gpsimd.collective_compute(
            "ReduceScatter", mybir.AluOpType.add, groups,
            ins=[part_t[:, :]], outs=[rsout_t[:, :]])

        # --- readback, normalize, store ---
        parts = [(rsout_t, 0, 128, 0), (rsout_t, 128, 128, 128),
                 (rsout_t, 256, 128, 256), (rsout_t, 384, 16, 384),
                 (rsslot_t, 0, SLOTS, BAGS)]
        for srcT, off, m, oo in parts:
            it = normp.tile([128, NW], bf16, tag="it")
            nc.sync.dma_start(it[0:m, :], srcT[off:off + m, :])
            rz = normp.tile([128, 1], f32, tag="rz")
            nc.vector.reciprocal(rz[0:m], it[0:m, D:D + 1])
            osb = normp.tile([128, D], f32, tag="osb")
            nc.vector.tensor_scalar(osb[0:m, :], it[0:m, 0:D], rz[0:m],
                                    None, mybir.AluOpType.mult)
            nc.sync.dma_start(out_d.ap()[oo:oo + m, :], osb[0:m, :])

    nc.compile()
    return nc


def _prep_shared(embed_table, W1, b1, W2):
    """Per-core-sliceable views of the table + tiny MLP weights."""
    t16 = embed_table.astype(np.float16)                      # [20000, 256]
    tabt = np.zeros((D, VP), np.float16)
    tabt[:, :NUM_CODE] = t16.T
    rhsc = np.zeros((VP, NW), np.float16)
    rhsc[:NUM_CODE, :D] = t16
    rhsc[:NUM_CODE, D] = 1.0
    w1t = np.ascontiguousarray(W1.astype(np.float16).T)       # [256, 128]
    w2c = np.ascontiguousarray(W2.astype(np.float16).reshape(H, 1))
    b1c = np.ascontiguousarray(b1.astype(np.float32).reshape(H, 1))
    return dict(tabt=tabt, rhsc=rhsc, w1t=w1t, w2c=w2c, b1=b1c)


def build_in_maps(input_code, length_code, shared):
    import ml_dtypes

    codes = input_code.reshape(GBAGS, C).astype(np.int64)
    lens = length_code.reshape(GBAGS).astype(np.int64)

    # global column order: slot tiles first (8 cores x 32), then 3200 bags
    # core-major (col = NSLOT + core*400 + i)
    NCOL = NSLOT + GBAGS
    cnt = np.zeros((VP, NCOL), np.float32)
    valid = np.arange(C)[None, :] < lens[:, None]
    bb, cc = np.nonzero(valid)
    np.add.at(cnt, (codes[bb, cc], NSLOT + bb), 1.0)

    len0_lists = []
    for core in range(NCORES):
        len0 = np.nonzero(lens[core * BAGS:(core + 1) * BAGS] == 0)[0][:SLOTS]
        len0_lists.append(len0)
        for s, b in enumerate(len0):
            np.add.at(cnt, (codes[core * BAGS + b],
                            core * SLOTS + s), 1.0)

    in_maps = []
    for core in range(NCORES):
        vs = slice(core * VSL, (core + 1) * VSL)
        cslice = cnt[vs]                                      # [2560, 3456]
        cntl = np.ascontiguousarray(
            cslice.reshape(NCH, 128, NT, 128).transpose(1, 2, 0, 3)
        ).astype(ml_dtypes.float8_e4m3).reshape(128, NT * NCH * 128)
        tabtc = np.ascontiguousarray(
            shared["tabt"][:, vs].reshape(2, 128, VSL).transpose(1, 0, 2)
        ).reshape(128, 2 * VSL)
        rhscc = np.ascontiguousarray(
            shared["rhsc"][vs].reshape(NCH, 128, NW).transpose(1, 0, 2)
        ).reshape(128, NCH * NW)
        in_maps.append(dict(tabt=tabtc, rhsc=rhscc, cnt=cntl,
                            w1t=shared["w1t"], w2c=shared["w2c"],
                            b1=shared["b1"]))
    return in_maps, len0_lists


def kernel(input_code, length_code, embed_table, W1, b1, W2, b2):
    from concourse.bass_utils import run_bass_kernel_spmd

    if "nc" not in _cache:
        _cache["nc"] = _build_program()
    nc = _cache["nc"]

    shared = _prep_shared(np.asarray(embed_table), np.asarray(W1),
                          np.asarray(b1), np.asarray(W2))
    in_maps, len0_lists = build_in_maps(np.asarray(input_code),
                                        np.asarray(length_code), shared)
    res = run_bass_kernel_spmd(nc, in_maps, core_ids=list(range(NCORES)))
    outs = []
    for c in range(NCORES):
        full = res.results[c]["out"]
        o = full[:BAGS].copy()
        for s, b in enumerate(len0_lists[c]):
            o[b] = full[BAGS + s]
        outs.append(o.reshape(BPC, V, D))
    return np.concatenate(outs, axis=0)
